# revision 18
# baseline (speedup 1.0000x reference)
"""Trainium2 Bass kernel for nn_LogisticModel.

Computes, for each batch row b:
    logp[b] = sum_t Normal(x_t - 0.9*x_{t-1} - sigmoid(s_t), 0.1).logpdf(0)
            = -0.5/0.01 * sum_t resid_t^2 + T * (-ln(0.1) - 0.5*ln(2*pi))
with x_{-1} = 0.  Pure elementwise + row reduction; sharded by batch rows
across 8 NeuronCores (512 rows per core).

The tolerance budget (rel err 2e-2 on a |logp| ~ 8.5e5 output) is enormous,
so inputs are downcast during the host-side shard step: s -> fp8 e4m3 (only
the ACT engine reads it, for sigmoid) and x -> bf16 (so DVE tensor ops get
the packed-16-bit fast modes).  HBM traffic per core drops 32 -> 12.6 MiB.

Per-chunk engine split (v = 0.9*x_prev - (x - sigmoid(s)) = -resid).
GpSimd compute is avoided (it shares SBUF ports with DVE; measured traces
show DVE tensor ops stall up to 20x while GpSimd streams) and
tensor_tensor_reduce crashes the exec unit on this runtime, so the square+
reduce is split between ACT and the otherwise-idle PE (tensor) engine:
    ACT : sigma = sigmoid(s)            (fp8 in, bf16 out)
    DVE : w = x - sigma                 (TT, bf16, 2x mode)
    DVE : pp = 0.9 * x_prev             (TS, bf16, 4x mode)
    DVE : v = pp - w                    (TT, bf16, 2x mode)
    ACT : Square(v[:, :C]) accum        (fused square+reduce, ~half)
    DMA : block-transpose v[:, C:] -> vT tiles   (xbar dma, free compute)
    PE  : Gram += vT_blk.T @ vT_blk accumulated in PSUM per row-group;
          diag(Gram)[p] = sum_t v[p, t]^2.  Diagonal extracted once per
          group with an eye mask + row reduce on DVE.

Chunk widths ramp small -> large -> small so the ACT/DVE pipeline fills
quickly after the first small DMA and drains quickly after the last one.

Self-contained: hardcodes B=4096, T=8192.
"""

import math
import sys

import ml_dtypes
import numpy as np

sys.path.insert(0, "/opt/trn_rl_repo")

import concourse.bacc as bacc  # noqa: E402
import concourse.tile as tile  # noqa: E402
from concourse import mybir  # noqa: E402
from concourse.bass_utils import run_bass_kernel_spmd  # noqa: E402

GAIN = 1.0
DECAY = 0.9
NOISE = 0.1
LOG_2PI = math.log(2.0 * math.pi)

B, T = 4096, 8192
N_CORES = 8
ROWS_PER_CORE = B // N_CORES          # 512
P = 128                               # SBUF partitions
N_GROUP = ROWS_PER_CORE // P          # 4 row-groups per core

C1 = -0.5 / (NOISE * NOISE)                      # -50.0
C2 = T * (-math.log(NOISE) - 0.5 * LOG_2PI)      # per-row additive constant

# Per-group chunk plans: (width, act_cols) pairs; act_cols go through the
# ACT Square path, the rest through the PE Gram path ((w - cb) % 128 == 0).
_PLAN_HEAD = [(1024, 512), (3072, 1536), (4096, 2048)]
_PLAN_TAIL = [(4096, 2048), (3072, 1536), (1024, 512)]

_cache = {}


def _build(bufs=3, xbufs=2):
    """Build and schedule the per-core Tile kernel (same program on all 8)."""
    nc = bacc.Bacc("TRN2", target_bir_lowering=False, debug=False,
                   num_devices=N_CORES)
    f32 = mybir.dt.float32
    bf16 = mybir.dt.bfloat16
    fp8 = mybir.dt.float8e4
    s_d = nc.dram_tensor("s", [ROWS_PER_CORE, T], fp8, kind="ExternalInput").ap()
    x_d = nc.dram_tensor("x", [ROWS_PER_CORE, T], bf16, kind="ExternalInput").ap()
    e_d = nc.dram_tensor("eye", [P, P], f32, kind="ExternalInput").ap()
    o_d = nc.dram_tensor("o", [P, N_GROUP], f32, kind="ExternalOutput").ap()

    Alu = mybir.AluOpType
    Act = mybir.ActivationFunctionType

    plans = [list(_PLAN_HEAD) for _ in range(N_GROUP - 1)] + [list(_PLAN_TAIL)]
    for pl in plans:
        assert sum(w for w, _ in pl) == T
        assert all((w - cb) % P == 0 for w, cb in pl)
    max_chunks = max(len(pl) for pl in plans)

    with tile.TileContext(nc) as tc:
        with (
            tc.tile_pool(name="xp", bufs=xbufs) as xp,
            tc.tile_pool(name="io", bufs=bufs) as io,
            tc.tile_pool(name="accp", bufs=1) as accp,
            tc.tile_pool(name="ps", bufs=2, space="PSUM") as ps,
        ):
            acc = accp.tile([P, N_GROUP, max_chunks], f32)    # ACT partials
            eye = accp.tile([P, P], f32)
            diag = accp.tile([P, P], f32)
            t_act = accp.tile([P, N_GROUP], f32)
            t_pe = accp.tile([P, N_GROUP], f32)
            logp = accp.tile([P, N_GROUP], f32)
            nc.vector.memset(acc[:], 0.0)
            nc.sync.dma_start(out=eye[:], in_=e_d[:, :])

            for g in range(N_GROUP):
                rows = slice(g * P, (g + 1) * P)
                plan = plans[g]
                n_blk_g = sum((w - cb) // P for w, cb in plan)
                # Whole-row x tile with one zero pad column at the front so
                # x_prev is just a shifted view (no overlap re-read).
                xx = xp.tile([P, T + 1], bf16, tag="xx")
                nc.vector.memset(xx[:, 0:1], 0.0)
                gram = ps.tile([P, P], f32, tag="gram")

                blk_i = 0
                col = 0
                for j, (w_, cb) in enumerate(plan):
                    nb = (w_ - cb) // P
                    s_t = io.tile([P, w_], fp8, tag="s")
                    sig = io.tile([P, w_], bf16, tag="sig")
                    w_t = io.tile([P, w_], bf16, tag="w")
                    pp = io.tile([P, w_], bf16, tag="pp")
                    v_t = io.tile([P, w_], bf16, tag="v")
                    vT = io.tile([P, nb, P], bf16, tag="vT")
                    junk = io.tile([P, cb], bf16, tag="junk")

                    nc.sync.dma_start(out=s_t[:], in_=s_d[rows, col:col + w_])
                    nc.sync.dma_start(out=xx[:, col + 1:col + w_ + 1],
                                      in_=x_d[rows, col:col + w_])
                    # sigma = sigmoid(GAIN * s)
                    nc.scalar.activation(out=sig[:], in_=s_t[:],
                                         func=Act.Sigmoid, scale=GAIN)
                    # w = x - sigma  (TT, bf16 -> 2x mode)
                    nc.vector.tensor_sub(w_t[:], xx[:, col + 1:col + w_ + 1],
                                         sig[:])
                    # pp = 0.9 * x_prev  (TS, bf16 -> 4x mode)
                    nc.vector.tensor_scalar(out=pp[:], in0=xx[:, col:col + w_],
                                            scalar1=DECAY, scalar2=None,
                                            op0=Alu.mult)
                    # v = pp - w = -resid  (sign dies in the square)
                    nc.vector.tensor_sub(v_t[:], pp[:], w_t[:])
                    # acc[:, g, j] = sum v[:, :cb]^2 on ACT
                    nc.scalar.activation(out=junk[:], in_=v_t[:, 0:cb],
                                         func=Act.Square,
                                         accum_out=acc[:, g, j:j + 1])
                    # PE path: block-transpose v[:, cb:], Gram-accumulate
                    nc.sync.dma_start_transpose(out=vT[:], in_=v_t[:, cb:w_])
                    for b in range(nb):
                        nc.tensor.matmul(gram[:], vT[:, b, :], vT[:, b, :],
                                         start=(blk_i == 0),
                                         stop=(blk_i == n_blk_g - 1))
                        blk_i += 1
                    col += w_

                # diag(Gram) = per-row sum of squares for the PE share
                nc.vector.tensor_mul(diag[:], gram[:], eye[:])
                nc.vector.tensor_reduce(out=t_pe[:, g:g + 1], in_=diag[:],
                                        axis=mybir.AxisListType.X, op=Alu.add)

            nc.vector.tensor_reduce(
                out=t_act[:], in_=acc[:],
                axis=mybir.AxisListType.X, op=Alu.add)
            nc.vector.tensor_add(logp[:], t_pe[:], t_act[:])
            nc.vector.tensor_scalar(
                out=logp[:], in0=logp[:], scalar1=C1, scalar2=C2,
                op0=Alu.mult, op1=Alu.add,
            )
            nc.sync.dma_start(out=o_d[:], in_=logp[:])

    nc.compile()
    return nc


def _run(s, x, trace=False, **build_kwargs):
    key = tuple(sorted(build_kwargs.items()))
    if key not in _cache:
        _cache[key] = _build(**build_kwargs)
    nc = _cache[key]

    s8 = np.asarray(s, dtype=np.float32).astype(ml_dtypes.float8_e4m3)
    x16 = np.asarray(x, dtype=np.float32).astype(ml_dtypes.bfloat16)
    eye = np.eye(P, dtype=np.float32)

    in_maps = []
    for k in range(N_CORES):
        r0 = k * ROWS_PER_CORE
        in_maps.append({
            "s": np.ascontiguousarray(s8[r0:r0 + ROWS_PER_CORE]),
            "x": np.ascontiguousarray(x16[r0:r0 + ROWS_PER_CORE]),
            "eye": eye,
        })

    res = run_bass_kernel_spmd(nc, in_maps, list(range(N_CORES)), trace=trace)

    out = np.empty((B,), dtype=np.float32)
    for k in range(N_CORES):
        # o[p, g] holds the row g*P + p of this core's shard
        out[k * ROWS_PER_CORE:(k + 1) * ROWS_PER_CORE] = (
            np.asarray(res.results[k]["o"]).T.reshape(-1)
        )
    return out, res


def kernel(s, x):
    out, _ = _run(np.asarray(s, dtype=np.float32), np.asarray(x, dtype=np.float32))
    return out


if __name__ == "__main__":
    rng = np.random.default_rng(0)
    s = rng.standard_normal((B, T), dtype=np.float32)
    x = rng.standard_normal((B, T), dtype=np.float32)
    out = kernel(s, x)
    print(out.shape, out.dtype, out[:4])


# revision 24
# speedup vs baseline: 1.1562x; 1.1562x over previous
"""Trainium2 Bass kernel for nn_LogisticModel.

Computes, for each batch row b:
    logp[b] = sum_t Normal(x_t - 0.9*x_{t-1} - sigmoid(s_t), 0.1).logpdf(0)
            = -0.5/0.01 * sum_t resid_t^2 + T * (-ln(0.1) - 0.5*ln(2*pi))
with x_{-1} = 0.  Pure elementwise + row reduction; sharded by batch rows
across 8 NeuronCores (512 rows per core).

The tolerance budget (rel err 2e-2 on a |logp| ~ 8.5e5 output) is enormous,
so inputs are downcast during the host-side shard step: s -> fp8 e4m3 (only
the ACT engine reads it, for sigmoid) and x -> bf16 (so DVE tensor ops get
the packed-16-bit fast modes).  HBM traffic per core drops 32 -> 12.6 MiB.

Per-chunk engine split (v = 0.9*x_prev - (x - sigmoid(s)) = -resid).
GpSimd compute is avoided (it shares SBUF ports with DVE; measured traces
show DVE tensor ops stall up to 20x while GpSimd streams) and
tensor_tensor_reduce crashes the exec unit on this runtime, so the square+
reduce is split between ACT and the otherwise-idle PE (tensor) engine:
    ACT : sigma = sigmoid(s)            (fp8 in, bf16 out)
    DVE : w = x - sigma                 (TT, bf16, 2x mode)
    DVE : pp = 0.9 * x_prev             (TS, bf16, 4x mode)
    DVE : v = pp - w                    (TT; ACT-share cols to a chunk tile,
          PE-share cols appended to a contiguous per-group tile vpe)
    ACT : Square(v[:, :C]) accum        (fused square+reduce)
    DMA : ONE xbar block-transpose of vpe per group -> vT tiles
    PE  : Gram += vT_blk.T @ vT_blk accumulated in PSUM per row-group;
          diag(Gram)[p] = sum_t v[p, t]^2, extracted with an eye mask +
          row reduce on DVE.

The transpose waits on the whole group's v, so on the in-order SP DMA
queue it is emitted AFTER the next group's input DMAs (otherwise it
blocks those prefetches and collapses the pipeline -- measured +35us).
The last group is all-ACT so no PE work trails the final chunk.

Self-contained: hardcodes B=4096, T=8192.
"""

import math
import sys

import ml_dtypes
import numpy as np

sys.path.insert(0, "/opt/trn_rl_repo")

import concourse.bacc as bacc  # noqa: E402
import concourse.tile as tile  # noqa: E402
from concourse import mybir  # noqa: E402
from concourse.bass_utils import run_bass_kernel_spmd  # noqa: E402

GAIN = 1.0
DECAY = 0.9
NOISE = 0.1
LOG_2PI = math.log(2.0 * math.pi)

B, T = 4096, 8192
N_CORES = 8
ROWS_PER_CORE = B // N_CORES          # 512
P = 128                               # SBUF partitions
N_GROUP = ROWS_PER_CORE // P          # 4 row-groups per core

C1 = -0.5 / (NOISE * NOISE)                      # -50.0
C2 = T * (-math.log(NOISE) - 0.5 * LOG_2PI)      # per-row additive constant

# Per-group chunk plans: (width, act_cols); act_cols go through ACT Square,
# the rest through the PE Gram path ((w - cb) % 128 == 0).  Last group is
# all-ACT so the PE pipeline never extends past the last compute chunk.
_PLAN_HEAD = [(1024, 512), (3072, 1536), (4096, 2048)]
_PLAN_LAST = [(4096, 4096), (3072, 3072), (1024, 1024)]

_cache = {}


def _build(bufs=3, xbufs=2):
    """Build and schedule the per-core Tile kernel (same program on all 8)."""
    nc = bacc.Bacc("TRN2", target_bir_lowering=False, debug=False,
                   num_devices=N_CORES)
    f32 = mybir.dt.float32
    bf16 = mybir.dt.bfloat16
    fp8 = mybir.dt.float8e4
    s_d = nc.dram_tensor("s", [ROWS_PER_CORE, T], fp8, kind="ExternalInput").ap()
    x_d = nc.dram_tensor("x", [ROWS_PER_CORE, T], bf16, kind="ExternalInput").ap()
    e_d = nc.dram_tensor("eye", [P, P], f32, kind="ExternalInput").ap()
    o_d = nc.dram_tensor("o", [P, N_GROUP], f32, kind="ExternalOutput").ap()

    Alu = mybir.AluOpType
    Act = mybir.ActivationFunctionType

    plans = [list(_PLAN_HEAD) for _ in range(N_GROUP - 1)] + [list(_PLAN_LAST)]
    for pl in plans:
        assert sum(w for w, _ in pl) == T
        assert all((w - cb) % P == 0 for w, cb in pl)
    max_chunks = max(len(pl) for pl in plans)
    pe_cols = [sum(w - cb for w, cb in pl) for pl in plans]

    with tile.TileContext(nc) as tc:
        with (
            tc.tile_pool(name="xp", bufs=xbufs) as xp,
            tc.tile_pool(name="io", bufs=bufs) as io,
            tc.tile_pool(name="accp", bufs=1) as accp,
            tc.tile_pool(name="ps", bufs=2, space="PSUM") as ps,
        ):
            acc = accp.tile([P, N_GROUP, max_chunks], f32)    # ACT partials
            eye = accp.tile([P, P], f32)
            diag = accp.tile([P, P], f32)
            junk = accp.tile([P, max(cb for pl in plans for _, cb in pl)],
                             bf16, name="junk")  # ACT Square scratch
            t_act = accp.tile([P, N_GROUP], f32)
            t_pe = accp.tile([P, N_GROUP], f32)
            logp = accp.tile([P, N_GROUP], f32)
            nc.vector.memset(acc[:], 0.0)
            nc.vector.memset(t_pe[:], 0.0)
            nc.sync.dma_start(out=eye[:], in_=e_d[:, :])

            group_state = {}

            def emit_pe_tail(g):
                """Transpose group g's vpe, Gram-accumulate on PE, extract
                the diagonal.  Called after group g+1's input DMAs so the
                transpose's data wait never blocks the SP DMA queue."""
                vpe, n_pe = group_state.pop(g)
                if n_pe == 0:
                    return
                nb = n_pe // P
                vT = xp.tile([P, nb, P], bf16, tag="vT")
                gram = ps.tile([P, P], f32, tag="gram")
                nc.sync.dma_start_transpose(out=vT[:], in_=vpe[:])
                for b in range(nb):
                    nc.tensor.matmul(gram[:], vT[:, b, :], vT[:, b, :],
                                     start=(b == 0), stop=(b == nb - 1))
                nc.vector.tensor_mul(diag[:], gram[:], eye[:])
                nc.vector.tensor_reduce(out=t_pe[:, g:g + 1], in_=diag[:],
                                        axis=mybir.AxisListType.X, op=Alu.add)

            for g in range(N_GROUP):
                rows = slice(g * P, (g + 1) * P)
                plan = plans[g]
                # Whole-row x tile with one zero pad column at the front so
                # x_prev is just a shifted view (no overlap re-read).
                xx = xp.tile([P, T + 1], bf16, tag="xx")
                nc.vector.memset(xx[:, 0:1], 0.0)
                col = 0
                s_tiles = []
                for j, (w_, cb) in enumerate(plan):
                    s_t = io.tile([P, w_], fp8, tag="s", name=f"s{g}_{j}")
                    s_tiles.append(s_t)
                    nc.sync.dma_start(out=s_t[:], in_=s_d[rows, col:col + w_])
                    nc.sync.dma_start(out=xx[:, col + 1:col + w_ + 1],
                                      in_=x_d[rows, col:col + w_])
                    col += w_

                if g > 0:
                    emit_pe_tail(g - 1)

                vpe = (xp.tile([P, pe_cols[g]], bf16, tag="vpe",
                               name=f"vpe{g}")
                       if pe_cols[g] else None)
                group_state[g] = (vpe, pe_cols[g])

                col = 0
                pe_off = 0
                for j, (w_, cb) in enumerate(plan):
                    s_t = s_tiles[j]
                    sig = io.tile([P, w_], bf16, tag="sig")
                    w_t = io.tile([P, w_], bf16, tag="w")
                    pp = io.tile([P, w_], bf16, tag="pp")
                    v_t = io.tile([P, cb], bf16, tag="v")

                    # sigma = sigmoid(GAIN * s)
                    nc.scalar.activation(out=sig[:], in_=s_t[:],
                                         func=Act.Sigmoid, scale=GAIN)
                    # w = x - sigma  (TT, bf16 -> 2x mode)
                    nc.vector.tensor_sub(w_t[:], xx[:, col + 1:col + w_ + 1],
                                         sig[:])
                    # pp = 0.9 * x_prev  (TS, bf16 -> 4x mode)
                    nc.vector.tensor_scalar(out=pp[:], in0=xx[:, col:col + w_],
                                            scalar1=DECAY, scalar2=None,
                                            op0=Alu.mult)
                    # v = pp - w = -resid  (sign dies in the square);
                    # ACT-share columns to v_t, PE-share columns into vpe.
                    nc.vector.tensor_sub(v_t[:], pp[:, 0:cb], w_t[:, 0:cb])
                    if w_ > cb:
                        nc.vector.tensor_sub(
                            vpe[:, pe_off:pe_off + (w_ - cb)],
                            pp[:, cb:w_], w_t[:, cb:w_])
                        pe_off += w_ - cb
                    # acc[:, g, j] = sum v[:, :cb]^2 on ACT
                    nc.scalar.activation(out=junk[:, 0:cb], in_=v_t[:],
                                         func=Act.Square,
                                         accum_out=acc[:, g, j:j + 1])
                    col += w_

            emit_pe_tail(N_GROUP - 1)

            nc.vector.tensor_reduce(
                out=t_act[:], in_=acc[:],
                axis=mybir.AxisListType.X, op=Alu.add)
            nc.vector.tensor_add(logp[:], t_pe[:], t_act[:])
            nc.vector.tensor_scalar(
                out=logp[:], in0=logp[:], scalar1=C1, scalar2=C2,
                op0=Alu.mult, op1=Alu.add,
            )
            nc.sync.dma_start(out=o_d[:], in_=logp[:])

    nc.compile()
    return nc


def _run(s, x, trace=False, **build_kwargs):
    key = tuple(sorted(build_kwargs.items()))
    if key not in _cache:
        _cache[key] = _build(**build_kwargs)
    nc = _cache[key]

    s8 = np.asarray(s, dtype=np.float32).astype(ml_dtypes.float8_e4m3)
    x16 = np.asarray(x, dtype=np.float32).astype(ml_dtypes.bfloat16)
    eye = np.eye(P, dtype=np.float32)

    in_maps = []
    for k in range(N_CORES):
        r0 = k * ROWS_PER_CORE
        in_maps.append({
            "s": np.ascontiguousarray(s8[r0:r0 + ROWS_PER_CORE]),
            "x": np.ascontiguousarray(x16[r0:r0 + ROWS_PER_CORE]),
            "eye": eye,
        })

    res = run_bass_kernel_spmd(nc, in_maps, list(range(N_CORES)), trace=trace)

    out = np.empty((B,), dtype=np.float32)
    for k in range(N_CORES):
        # o[p, g] holds the row g*P + p of this core's shard
        out[k * ROWS_PER_CORE:(k + 1) * ROWS_PER_CORE] = (
            np.asarray(res.results[k]["o"]).T.reshape(-1)
        )
    return out, res


def kernel(s, x):
    out, _ = _run(np.asarray(s, dtype=np.float32), np.asarray(x, dtype=np.float32))
    return out


if __name__ == "__main__":
    rng = np.random.default_rng(0)
    s = rng.standard_normal((B, T), dtype=np.float32)
    x = rng.standard_normal((B, T), dtype=np.float32)
    out = kernel(s, x)
    print(out.shape, out.dtype, out[:4])


# revision 25
# speedup vs baseline: 1.4511x; 1.2551x over previous
"""Trainium2 Bass kernel for nn_LogisticModel.

Computes, for each batch row b:
    logp[b] = sum_t Normal(x_t - 0.9*x_{t-1} - sigmoid(s_t), 0.1).logpdf(0)
            = -0.5/0.01 * sum_t resid_t^2 + T * (-ln(0.1) - 0.5*ln(2*pi))
with x_{-1} = 0.  Pure elementwise + row reduction; sharded by batch rows
across 8 NeuronCores (512 rows per core).

The tolerance budget (rel err 2e-2 on a |logp| ~ 8.5e5 output) is enormous,
so inputs are downcast during the host-side shard step: s -> fp8 e4m3 (only
the ACT engine reads it, for sigmoid) and x -> bf16 (so DVE tensor ops get
the packed-16-bit fast modes).  HBM traffic per core drops 32 -> 12.6 MiB.

Per-chunk engine split (v = 0.9*x_prev - (x - sigmoid(s)) = -resid).
GpSimd compute is avoided entirely: it shares SBUF ports with DVE, and
measured traces show DVE tensor ops stall up to 20x while GpSimd streams.
tensor_tensor_reduce crashes the exec unit on this runtime, so DVE's
share of the reduce uses bn_stats:
    ACT : sigma = sigmoid(s)            (fp8 in, bf16 out)
    DVE : w = x - sigma                 (TT, bf16, 2x mode)
    DVE : pp = 0.9 * x_prev             (TS, bf16, 4x mode)
    DVE : v = pp - w                    (TT, bf16, 2x mode)
    ACT : Square(v[:, :C]) accum        (fused square+reduce, ~73% of cols)
    DVE : bn_stats per 512-col block of v[:, C:]; sum v^2 recovered in the
          tail as M2_e + M2_o + 256*(mean_e^2 + mean_o^2) per block.

Chunk widths ramp small -> large -> small so the ACT/DVE pipeline fills
quickly after the first small DMA and drains quickly after the last one.

Self-contained: hardcodes B=4096, T=8192.
"""

import math
import sys

import ml_dtypes
import numpy as np

sys.path.insert(0, "/opt/trn_rl_repo")

import concourse.bacc as bacc  # noqa: E402
import concourse.tile as tile  # noqa: E402
from concourse import mybir  # noqa: E402
from concourse.bass_utils import run_bass_kernel_spmd  # noqa: E402

GAIN = 1.0
DECAY = 0.9
NOISE = 0.1
LOG_2PI = math.log(2.0 * math.pi)

B, T = 4096, 8192
N_CORES = 8
ROWS_PER_CORE = B // N_CORES          # 512
P = 128                               # SBUF partitions
N_GROUP = ROWS_PER_CORE // P          # 4 row-groups per core
BLK = 512                             # bn_stats hardware block limit

C1 = -0.5 / (NOISE * NOISE)                      # -50.0
C2 = T * (-math.log(NOISE) - 0.5 * LOG_2PI)      # per-row additive constant

# Per-group chunk plans: (width, act_cols) pairs; act_cols go through the
# ACT Square path, the rest through DVE bn_stats ((w - cb) % BLK == 0).
# First group starts small (fast pipeline fill), last group ends small
# (fast drain).  Aggregate ACT fraction ~0.81: bn_stats costs ~1.16ns/elem
# on DVE vs 0.83 on ACT, so ACT leans heavier.
_PLAN_A = [(1024, 512), (3072, 2560), (4096, 3584)]
_PLAN_B = [(1024, 512), (3072, 2560), (4096, 3072)]
_PLAN_TAIL = [(4096, 3072), (3072, 2560), (1024, 512)]

_cache = {}


def _build(bufs=3, xbufs=2):
    """Build and schedule the per-core Tile kernel (same program on all 8)."""
    nc = bacc.Bacc("TRN2", target_bir_lowering=False, debug=False,
                   num_devices=N_CORES)
    f32 = mybir.dt.float32
    bf16 = mybir.dt.bfloat16
    fp8 = mybir.dt.float8e4
    s_d = nc.dram_tensor("s", [ROWS_PER_CORE, T], fp8, kind="ExternalInput").ap()
    x_d = nc.dram_tensor("x", [ROWS_PER_CORE, T], bf16, kind="ExternalInput").ap()
    o_d = nc.dram_tensor("o", [P, N_GROUP], f32, kind="ExternalOutput").ap()

    Alu = mybir.AluOpType
    Act = mybir.ActivationFunctionType

    plans = [list(_PLAN_A), list(_PLAN_A), list(_PLAN_B), list(_PLAN_TAIL)]
    for pl in plans:
        assert sum(w for w, _ in pl) == T
        assert all((w - cb) % BLK == 0 for w, cb in pl)
    max_chunks = max(len(pl) for pl in plans)
    max_blk = max(sum((w - cb) // BLK for w, cb in pl) for pl in plans)

    with tile.TileContext(nc) as tc:
        with (
            tc.tile_pool(name="xp", bufs=xbufs) as xp,
            tc.tile_pool(name="io", bufs=bufs) as io,
            tc.tile_pool(name="accp", bufs=1) as accp,
        ):
            acc = accp.tile([P, N_GROUP, max_chunks], f32)    # ACT partials
            stats = accp.tile([P, N_GROUP, max_blk, 6], f32)  # bn_stats out
            me = accp.tile([P, N_GROUP, max_blk, 1], f32)
            mo = accp.tile([P, N_GROUP, max_blk, 1], f32)
            m2 = accp.tile([P, N_GROUP, max_blk, 1], f32)
            bs = accp.tile([P, N_GROUP, max_blk, 1], f32)
            t_act = accp.tile([P, N_GROUP], f32)
            t_dve = accp.tile([P, N_GROUP], f32)
            logp = accp.tile([P, N_GROUP], f32)
            nc.vector.memset(acc[:], 0.0)
            nc.vector.memset(stats[:], 0.0)

            for g in range(N_GROUP):
                rows = slice(g * P, (g + 1) * P)
                plan = plans[g]
                # Whole-row x tile with one zero pad column at the front so
                # x_prev is just a shifted view (no overlap re-read).
                xx = xp.tile([P, T + 1], bf16, tag="xx")
                nc.vector.memset(xx[:, 0:1], 0.0)

                blk_i = 0
                col = 0
                for j, (w_, cb) in enumerate(plan):
                    s_t = io.tile([P, w_], fp8, tag="s")
                    sig = io.tile([P, w_], bf16, tag="sig")
                    w_t = io.tile([P, w_], bf16, tag="w")
                    pp = io.tile([P, w_], bf16, tag="pp")
                    v_t = io.tile([P, w_], bf16, tag="v")
                    junk = io.tile([P, cb], bf16, tag="junk")

                    nc.sync.dma_start(out=s_t[:], in_=s_d[rows, col:col + w_])
                    nc.sync.dma_start(out=xx[:, col + 1:col + w_ + 1],
                                      in_=x_d[rows, col:col + w_])
                    # sigma = sigmoid(GAIN * s)
                    nc.scalar.activation(out=sig[:], in_=s_t[:],
                                         func=Act.Sigmoid, scale=GAIN)
                    # w = x - sigma  (TT, bf16 -> 2x mode)
                    nc.vector.tensor_sub(w_t[:], xx[:, col + 1:col + w_ + 1],
                                         sig[:])
                    # pp = 0.9 * x_prev  (TS, bf16 -> 4x mode)
                    nc.vector.tensor_scalar(out=pp[:], in0=xx[:, col:col + w_],
                                            scalar1=DECAY, scalar2=None,
                                            op0=Alu.mult)
                    # v = pp - w = -resid  (sign dies in the square)
                    nc.vector.tensor_sub(v_t[:], pp[:], w_t[:])
                    # acc[:, g, j] = sum v[:, :cb]^2 on ACT
                    nc.scalar.activation(out=junk[:], in_=v_t[:, 0:cb],
                                         func=Act.Square,
                                         accum_out=acc[:, g, j:j + 1])
                    # per-512 stats of v[:, cb:] on DVE
                    for bkt in range((w_ - cb) // BLK):
                        c0 = cb + bkt * BLK
                        nc.vector.bn_stats(
                            stats[:, g, blk_i, :], v_t[:, c0:c0 + BLK])
                        blk_i += 1
                    col += w_

            # tail: recover sum v^2 from bn_stats, fold with ACT partials
            nc.vector.tensor_mul(me[:], stats[:, :, :, 1:2],
                                 stats[:, :, :, 1:2])
            nc.vector.tensor_mul(mo[:], stats[:, :, :, 4:5],
                                 stats[:, :, :, 4:5])
            nc.vector.tensor_add(me[:], me[:], mo[:])
            nc.vector.tensor_add(m2[:], stats[:, :, :, 2:3],
                                 stats[:, :, :, 5:6])
            # bs = (BLK/2) * (mean_e^2 + mean_o^2) + (M2_e + M2_o)
            nc.vector.scalar_tensor_tensor(
                out=bs[:], in0=me[:], scalar=BLK / 2, in1=m2[:],
                op0=Alu.mult, op1=Alu.add)
            nc.vector.tensor_reduce(
                out=t_dve[:], in_=bs[:, :, :, 0],
                axis=mybir.AxisListType.X, op=Alu.add)
            nc.vector.tensor_reduce(
                out=t_act[:], in_=acc[:],
                axis=mybir.AxisListType.X, op=Alu.add)
            nc.vector.tensor_add(logp[:], t_dve[:], t_act[:])
            nc.vector.tensor_scalar(
                out=logp[:], in0=logp[:], scalar1=C1, scalar2=C2,
                op0=Alu.mult, op1=Alu.add,
            )
            nc.sync.dma_start(out=o_d[:], in_=logp[:])

    nc.compile()
    return nc


def _run(s, x, trace=False, **build_kwargs):
    key = tuple(sorted(build_kwargs.items()))
    if key not in _cache:
        _cache[key] = _build(**build_kwargs)
    nc = _cache[key]

    s8 = np.asarray(s, dtype=np.float32).astype(ml_dtypes.float8_e4m3)
    x16 = np.asarray(x, dtype=np.float32).astype(ml_dtypes.bfloat16)

    in_maps = []
    for k in range(N_CORES):
        r0 = k * ROWS_PER_CORE
        in_maps.append({
            "s": np.ascontiguousarray(s8[r0:r0 + ROWS_PER_CORE]),
            "x": np.ascontiguousarray(x16[r0:r0 + ROWS_PER_CORE]),
        })

    res = run_bass_kernel_spmd(nc, in_maps, list(range(N_CORES)), trace=trace)

    out = np.empty((B,), dtype=np.float32)
    for k in range(N_CORES):
        # o[p, g] holds the row g*P + p of this core's shard
        out[k * ROWS_PER_CORE:(k + 1) * ROWS_PER_CORE] = (
            np.asarray(res.results[k]["o"]).T.reshape(-1)
        )
    return out, res


def kernel(s, x):
    out, _ = _run(np.asarray(s, dtype=np.float32), np.asarray(x, dtype=np.float32))
    return out


if __name__ == "__main__":
    rng = np.random.default_rng(0)
    s = rng.standard_normal((B, T), dtype=np.float32)
    x = rng.standard_normal((B, T), dtype=np.float32)
    out = kernel(s, x)
    print(out.shape, out.dtype, out[:4])
